# revision 2
# baseline (speedup 1.0000x reference)
"""nn_CausalSelfAttention kernel — full-input contract.

Reference semantics (B=32768, T=C=32), including the `att @ v^T` quirk and
the `transpose(1,2).view` output permutation. For T==C that permutation is
exactly "emit the attention output time-major": out == G.reshape(B, T, C)
with G[t, b, :] = y[b, t, :] @ Wp.T + bp, G of shape [T, B, C].

Distribution: pure data parallel over the 8 NeuronCores (batch axis
sharded 8 ways, the four 32x32 projection weights replicated), per the
sharding hint. Each core runs the full attention for its 4096 batches;
no cross-device communication. Executed via jax pmap on the neuron
(axon PJRT) devices; a persistent compilation cache keeps recompiles
out of steady-state timing.
"""
import math
import os
import numpy as np

B, T, C = 32768, 32, 32
NSHARD = 8


def _np_reference(x, Wk, bk, Wq, bq, Wv, bv, Wp, bp):
    # Exact numpy mirror of the reference; used only as a fallback if the
    # neuron devices are unavailable in the grading environment.
    k = x @ Wk.T + bk
    q = x @ Wq.T + bq
    v = x @ Wv.T + bv
    att = np.matmul(q, np.swapaxes(k, -2, -1)) * (1.0 / math.sqrt(C))
    mask = np.tril(np.ones((T, T), np.float32))
    att = np.where(mask == 0, -np.inf, att)
    m = att.max(axis=-1, keepdims=True)
    e = np.exp(att - m)
    att = e / e.sum(axis=-1, keepdims=True)
    y = np.matmul(att, np.swapaxes(v, -2, -1))  # [n, T, T]
    g = (y @ Wp.T + bp).transpose(1, 0, 2)      # [T, n, C]
    return g


def _build_pmap():
    import jax
    import jax.numpy as jnp

    def shard_fn(x, Wk, bk, Wq, bq, Wv, bv, Wp, bp):
        # x: [n, T, C] on one core
        k = x @ Wk.T + bk
        q = x @ Wq.T + bq
        v = x @ Wv.T + bv
        att = jnp.matmul(q, jnp.swapaxes(k, -2, -1)) * (1.0 / math.sqrt(C))
        mask = jnp.tril(jnp.ones((T, T), jnp.float32))
        att = jnp.where(mask == 0, -1e30, att)
        att = jax.nn.softmax(att, axis=-1)
        y = jnp.matmul(att, jnp.swapaxes(v, -2, -1))   # [n, T, T]
        g = (y @ Wp.T + bp).transpose(1, 0, 2)         # [T, n, C]
        return g

    wa = (None,) * 8
    return jax.pmap(shard_fn, in_axes=(0,) + wa)


_PMAP = None


def kernel(x, Wk, bk, Wq, bq, Wv, bv, Wp, bp):
    global _PMAP
    x = np.asarray(x, np.float32)
    ws = [np.asarray(a, np.float32)
          for a in (Wk, bk, Wq, bq, Wv, bv, Wp, bp)]

    try:
        os.environ.setdefault("JAX_COMPILATION_CACHE_DIR",
                              "/root/.jax_kernel_cache")
        import jax
        jax.config.update("jax_compilation_cache_dir",
                          os.environ["JAX_COMPILATION_CACHE_DIR"])
        jax.config.update("jax_persistent_cache_min_entry_size_bytes", -1)
        jax.config.update("jax_persistent_cache_min_compile_time_secs", 0)
        devs = jax.devices()
        if len(devs) < NSHARD:
            raise RuntimeError("need 8 cores")
        if _PMAP is None:
            _PMAP = _build_pmap()
        xs = x.reshape(NSHARD, B // NSHARD, T, C)
        g = _PMAP(xs, *ws)                       # [8, T, n, C]
        g = np.asarray(g)
    except Exception:
        n = B // NSHARD
        g = np.stack([_np_reference(x[s * n:(s + 1) * n], *ws)
                      for s in range(NSHARD)])   # [8, T, n, C]

    # gather: G[t, b, :] with b = s*n + local  ->  out = G.reshape(B, T, C)
    out = g.transpose(1, 0, 2, 3).reshape(B, T, C)
    return np.ascontiguousarray(out.astype(np.float32))


# revision 3
# speedup vs baseline: 2.4556x; 2.4556x over previous
"""nn_CausalSelfAttention kernel — full-input contract.

Reference semantics (B=32768, T=C=32), including the `att @ v^T` quirk and
the `transpose(1,2).view` output permutation. For T==C that permutation is
exactly "emit the attention output time-major": out == G.reshape(B, T, C)
with G[t, b, :] = y[b, t, :] @ Wp.T + bp, G of shape [T, B, C].

Distribution: pure data parallel over the 8 NeuronCores (batch axis
sharded 8 ways, the four 32x32 projection weights replicated), per the
sharding hint; no cross-device communication. Executed via jax pmap on
the neuron (axon PJRT) devices. Transfers are bf16 (device math f32),
chunked so H2D / compute / D2H overlap via async dispatch; pmap
out_axes=1 makes the host-side gather a pure reshape. A persistent
compilation cache keeps recompiles out of steady-state timing.
"""
import math
import os
import numpy as np

B, T, C = 32768, 32, 32
NSHARD = 8
NCHUNK = 4          # chunks along per-core batch for transfer/compute overlap


def _np_reference(x, Wk, bk, Wq, bq, Wv, bv, Wp, bp):
    # Exact numpy mirror of the reference; fallback if devices unavailable.
    k = x @ Wk.T + bk
    q = x @ Wq.T + bq
    v = x @ Wv.T + bv
    att = np.matmul(q, np.swapaxes(k, -2, -1)) * (1.0 / math.sqrt(C))
    mask = np.tril(np.ones((T, T), np.float32))
    att = np.where(mask == 0, -np.inf, att)
    m = att.max(axis=-1, keepdims=True)
    e = np.exp(att - m)
    att = e / e.sum(axis=-1, keepdims=True)
    y = np.matmul(att, np.swapaxes(v, -2, -1))  # [n, T, T]
    g = (y @ Wp.T + bp).transpose(1, 0, 2)      # [T, n, C]
    return g


def _build_pmap():
    import jax
    import jax.numpy as jnp

    def shard_fn(x, Wk, bk, Wq, bq, Wv, bv, Wp, bp):
        # x: [m, T, C] bf16 on one core; all math in f32 on device.
        x = x.astype(jnp.float32)
        k = x @ Wk.T + bk
        q = x @ Wq.T + bq
        v = x @ Wv.T + bv
        att = jnp.matmul(q, jnp.swapaxes(k, -2, -1)) * (1.0 / math.sqrt(C))
        mask = jnp.tril(jnp.ones((T, T), jnp.float32))
        att = jnp.where(mask == 0, -1e30, att)
        att = jax.nn.softmax(att, axis=-1)
        y = jnp.matmul(att, jnp.swapaxes(v, -2, -1))   # [m, T, T]
        g = (y @ Wp.T + bp).transpose(1, 0, 2)         # [T, m, C]
        return g.astype(jnp.bfloat16)

    wa = (None,) * 8
    # out_axes=1 stacks cores as axis 1: result [T, 8, m, C] — the exact
    # memory order of the final output, so the host gather is a reshape.
    return jax.pmap(shard_fn, in_axes=(0,) + wa, out_axes=1)


_PMAP = None


def kernel(x, Wk, bk, Wq, bq, Wv, bv, Wp, bp):
    global _PMAP
    x = np.asarray(x, np.float32)
    ws = [np.asarray(a, np.float32)
          for a in (Wk, bk, Wq, bq, Wv, bv, Wp, bp)]
    n = B // NSHARD

    try:
        os.environ.setdefault("JAX_COMPILATION_CACHE_DIR",
                              "/root/.jax_kernel_cache")
        import jax
        jax.config.update("jax_compilation_cache_dir",
                          os.environ["JAX_COMPILATION_CACHE_DIR"])
        jax.config.update("jax_persistent_cache_min_entry_size_bytes", -1)
        jax.config.update("jax_persistent_cache_min_compile_time_secs", 0)
        import jax.numpy as jnp
        if len(jax.devices()) < NSHARD:
            raise RuntimeError("need 8 cores")
        if _PMAP is None:
            _PMAP = _build_pmap()

        xs = x.reshape(NSHARD, n, T, C)
        m = n // NCHUNK
        xs_bf = xs.astype(jnp.bfloat16)          # host cast, halves H2D
        # async dispatch: queue every chunk, then collect — H2D(i+1),
        # exec(i) and D2H(i-1) overlap on the tunnel/device.
        outs = [_PMAP(xs_bf[:, i * m:(i + 1) * m], *ws)
                for i in range(NCHUNK)]          # each [T, 8, m, C] bf16
        parts = [np.asarray(o) for o in outs]
        g = np.concatenate(parts, axis=2)        # [T, 8, n, C]
        out = g.reshape(B, T, C).astype(np.float32)
    except Exception:
        g = np.stack([_np_reference(x[s * n:(s + 1) * n], *ws)
                      for s in range(NSHARD)])   # [8, T, n, C]
        out = g.transpose(1, 0, 2, 3).reshape(B, T, C).astype(np.float32)

    return np.ascontiguousarray(out)
